# revision 9
# baseline (speedup 1.0000x reference)
"""Trainium2 Bass kernel for nn_Char_30322469110372 (retrieval_knn).

Reference computation (per query b):
  ce   = row-normalized ce_raw (+ zero pad row for index -1)
  q    = ce[qidx[b]]
  for side in (l, r):
    u_side      = W_side @ q                     # [C]
    score[k]    = ce[ixs_c[b,k]] . u_side        # masked to -1e30 where ixs==-1
    attn        = softmax(score)
    emb_side    = sum_k attn[k] * wvec[ixs_w[b,k]]
  gate = softmax([attn_l, attn_r] @ gL_w.T + gL_b)
  out  = gate[0]*emb_l + gate[1]*emb_r

Sharding: data-parallel over B across 8 cores; ce/wvec tables replicated.

The binding constraint is the Pool engine's SWDGE: each indirect-DMA gather
instruction costs ~1us fixed (994ns + 0.34ns/descriptor) and fetches at most
128 rows (one index per partition; multi-index offset APs mis-encode on this
runtime -- verified empirically, the ucode receives a garbage descriptor
layout). So runtime ~= (#gather instructions) x 1.04us, Pool-serial.

To cut the instruction count, exploit the trailing '<pad>' structure: per
query only llen in [1,7] left and rlen in [1,10] right slots are valid, and
pad slots carry exactly-zero softmax weight. kernel() sorts queries by
(llen, rlen) on the host (index marshalling only), deals them round-robin to
the 8 cores so every core's tile t has an identical length profile, and
compiles the device program with per-tile slot counts lmax[t]/rmax[t] =
the max valid lengths in that tile. Slots >= the tile max are all-pad for
every query in the tile (so the existing -1e30 mask already covers them) and
their ce/wvec gather instructions, dot products, and accumulation steps are
dropped entirely. This removes ~35% of gather instructions (~560 -> ~370).

Device algorithm per core (B_core=2048 queries, 16 tiles of 128 queries,
processed in chunks of 4 tiles so gather DMA, DVE, ACT and Pool overlap):
  - normalization folded into scores: score = (ctx_raw.u_raw) * rctx * rq with
    rctx/rq = 1/max(||row||,1e-12) computed on gathered rows only.
  - scores are bounded (|score| <= ~1.2) so softmax needs no max-shift;
    exp(-1e30) underflows to exactly 0 for pad slots.
  - per tile: indirect-DMA gather of 1+lmax+rmax ce rows/query; PE transposes
    q and computes u = [qT]^T @ [lW^T | rW^T]; DVE does the dot products; ACT
    squares rows, DVE reduces to row sum-squares. dotraw/cssq staging is
    memset to 0 once so ungathered slots stay finite (then masked).
  - per chunk: softmax + gate pipeline on [128, 4*17] staging buffers.
  - per tile: gather lmax+rmax wvec rows/query (pad slots clamped to row 0;
    their weight is exactly 0 and the clamp keeps every gathered pool slot
    fully populated, so no NaN can enter the accumulation), then a
    scalar*tensor+tensor accumulation chain on DVE over the gathered slots.
"""

from contextlib import ExitStack

import numpy as np

import concourse.bacc as bacc
import concourse.bass as bass
import concourse.mybir as mybir
import concourse.tile as tile
from concourse.bass_utils import run_bass_kernel_spmd
from concourse.masks import make_identity

# Problem shapes (hardcoded per contest contract).
P = 128
CD = 100          # char-embedding dim
L, R = 7, 10
K = L + R         # 17 context slots per query
KQ = K + 1        # + the query row itself
NCE = 200000      # ce table rows
V = 200000        # wvec table rows
WD = 300          # word-vector dim
B = 16384
N_CORES = 8
BC = B // N_CORES     # 2048 queries per core
NT = BC // P          # 16 tiles of 128 queries
CHUNK = 4             # tiles per phase chunk
OOB = 1 << 22         # stand-in index for -1; fails the DMA bounds check
WVP_BUFS = 3          # wv gather tile pool depth

F32 = mybir.dt.float32
I32 = mybir.dt.int32
Alu = mybir.AluOpType
Act = mybir.ActivationFunctionType
Ax = mybir.AxisListType


def _build_nc(lmax, rmax):
    """Build the SPMD program for per-tile valid-slot counts lmax/rmax."""
    nc = bacc.Bacc("TRN2", target_bir_lowering=False, debug=False,
                   num_devices=N_CORES)

    ce = nc.dram_tensor("ce_raw", [NCE, CD], F32, kind="ExternalInput")
    wv = nc.dram_tensor("wvec", [V, WD], F32, kind="ExternalInput")
    lW = nc.dram_tensor("lW", [CD, CD], F32, kind="ExternalInput")
    rW = nc.dram_tensor("rW", [CD, CD], F32, kind="ExternalInput")
    gw = nc.dram_tensor("gL_w", [2, K], F32, kind="ExternalInput")
    gb = nc.dram_tensor("gL_b", [2], F32, kind="ExternalInput")
    qidx = nc.dram_tensor("qidx", [BC], I32, kind="ExternalInput")
    lic = nc.dram_tensor("lixs_c", [BC, L], I32, kind="ExternalInput")
    ric = nc.dram_tensor("rixs_c", [BC, R], I32, kind="ExternalInput")
    liw = nc.dram_tensor("lixs_w", [BC, L], I32, kind="ExternalInput")
    riw = nc.dram_tensor("rixs_w", [BC, R], I32, kind="ExternalInput")
    out = nc.dram_tensor("out", [BC, WD], F32, kind="ExternalOutput")

    with tile.TileContext(nc) as tc, ExitStack() as ctx:
        consts = ctx.enter_context(tc.tile_pool(name="consts", bufs=1))
        stage = ctx.enter_context(tc.tile_pool(name="stage", bufs=1))
        cep = ctx.enter_context(tc.tile_pool(name="cep", bufs=4))
        wvp = ctx.enter_context(tc.tile_pool(name="wvp", bufs=WVP_BUFS))
        work = ctx.enter_context(tc.tile_pool(name="work", bufs=2))
        psum = ctx.enter_context(tc.tile_pool(name="psum", bufs=2, space="PSUM"))

        # ---------------- constants ----------------
        identity = consts.tile([P, P], F32)
        make_identity(nc, identity[:, :])

        # W^T for both sides packed as [100, 0:100]=lW^T, [100, 100:200]=rW^T
        wt_both = consts.tile([P, 2 * CD], F32)
        for side, wdram in enumerate((lW, rW)):
            wl = consts.tile([P, P], F32, name=f"wload{side}")
            nc.sync.dma_start(out=wl[0:CD, 0:CD], in_=wdram[:, :])
            wt_ps = psum.tile([P, P], F32, name=f"wt_ps{side}", tag="wt_ps")
            nc.tensor.transpose(
                out=wt_ps[0:CD, 0:CD], in_=wl[0:CD, 0:CD],
                identity=identity[0:CD, 0:CD])
            nc.vector.tensor_copy(
                out=wt_both[0:CD, side * CD:(side + 1) * CD],
                in_=wt_ps[0:CD, 0:CD])

        # gate weights replicated across partitions via PE outer product
        # (ones[128,1] @ row[1,36]); gwrep[:, j*K+k] = gL_w[j,k], cols 34:36=gL_b
        gwrow = consts.tile([1, 2 * K + 2], F32)
        nc.sync.dma_start(out=gwrow[0:1, 0:2 * K], in_=gw[:, :])
        nc.sync.dma_start(out=gwrow[0:1, 2 * K:2 * K + 2], in_=gb[:])
        ones1 = consts.tile([1, P], F32)
        nc.gpsimd.memset(ones1[:, :], 1.0)
        rep_ps = psum.tile([P, 2 * K + 2], F32, tag="rep_ps")
        nc.tensor.matmul(out=rep_ps[:, :], lhsT=ones1[0:1, :],
                         rhs=gwrow[0:1, :], start=True, stop=True)
        gwrep = consts.tile([P, 2 * K + 2], F32)
        nc.vector.tensor_copy(out=gwrep[:, :], in_=rep_ps[:, :])
        gbd = consts.tile([P, 1], F32)
        nc.vector.tensor_tensor(
            out=gbd[:, :], in0=gwrep[:, 2 * K + 1:2 * K + 2],
            in1=gwrep[:, 2 * K:2 * K + 1], op=Alu.subtract)

        # ---------------- index staging ----------------
        # cidx[p, t, 0] = qidx, [p, t, 1:8] = lixs_c, [p, t, 8:18] = rixs_c
        cidx = stage.tile([P, NT, KQ], I32)
        nc.sync.dma_start(out=cidx[:, :, 0],
                          in_=qidx[:].rearrange("(t p) -> p t", p=P))
        nc.sync.dma_start(out=cidx[:, :, 1:1 + L],
                          in_=lic[:, :].rearrange("(t p) k -> p t k", p=P))
        nc.sync.dma_start(out=cidx[:, :, 1 + L:KQ],
                          in_=ric[:, :].rearrange("(t p) k -> p t k", p=P))
        widx = stage.tile([P, NT, K], I32)
        nc.sync.dma_start(out=widx[:, :, 0:L],
                          in_=liw[:, :].rearrange("(t p) k -> p t k", p=P))
        nc.sync.dma_start(out=widx[:, :, L:K],
                          in_=riw[:, :].rearrange("(t p) k -> p t k", p=P))

        # clamped ce indices (pad -> row 0; masked later)
        ccl = stage.tile([P, NT, KQ], I32)
        nc.vector.tensor_scalar(out=ccl[:, :, :], in0=cidx[:, :, :],
                                scalar1=0, scalar2=None, op0=Alu.max)
        # wv indices clamped (pad -> row 0; weight is exactly 0 so the row 0
        # data never contributes). Clamp-always keeps every gathered pool
        # slot fully populated (no stale-data hazards) and keeps the Pool
        # instruction free of the bounds-check register input.
        wcl = stage.tile([P, NT, K], I32)
        nc.vector.tensor_scalar(out=wcl[:, :, :], in0=widx[:, :, :],
                                scalar1=0, scalar2=None, op0=Alu.max)

        # additive score mask: 0 for valid slots, -1e30 where index == -1.
        # Slots >= lmax/rmax are all-pad within the tile, so the mask also
        # covers every slot whose gather instruction we drop.
        maskf = stage.tile([P, NT, KQ], F32)
        nc.vector.tensor_copy(out=maskf[:, :, :], in_=cidx[:, :, :])
        maskt = stage.tile([P, NT, KQ], F32)
        nc.vector.tensor_scalar(out=maskt[:, :, :], in0=maskf[:, :, :],
                                scalar1=0.0, scalar2=1e30,
                                op0=Alu.min, op1=Alu.mult)

        # ---------------- staging buffers ----------------
        # dotraw/cssq are zero-filled so slots never written (ungathered)
        # stay finite; their scores end up 0 and are then masked to -1e30.
        dotraw = stage.tile([P, NT, K], F32)
        nc.vector.memset(dotraw[:, :, :], 0.0)
        cssq = stage.tile([P, NT, K], F32)
        nc.vector.memset(cssq[:, :, :], 0.0)
        qssq = stage.tile([P, NT], F32)
        expv = stage.tile([P, NT, K], F32)
        wall = stage.tile([P, NT, K], F32)

        def phase1_tile(t):
            lm, rm = lmax[t], rmax[t]
            ceg = cep.tile([P, KQ, CD], F32, name="ceg", tag="ceg")
            slots = [0] + list(range(1, 1 + lm)) + list(range(1 + L, 1 + L + rm))
            for s in slots:
                nc.gpsimd.indirect_dma_start(
                    out=ceg[:, s, :], out_offset=None,
                    in_=ce[:, :],
                    in_offset=bass.IndirectOffsetOnAxis(
                        ap=ccl[:, t, s:s + 1], axis=0),
                )
            # transpose q rows: qt = ceg[:, 0, :]^T  -> [CD, P]
            qt_ps = psum.tile([P, P], F32, name="qt_ps", tag="qt_ps")
            nc.tensor.transpose(out=qt_ps[0:CD, :], in_=ceg[:, 0, :],
                                identity=identity[:, :])
            qt_sb = work.tile([P, P], F32, name="qt_sb", tag="qt_sb")
            nc.scalar.copy(out=qt_sb[0:CD, :], in_=qt_ps[0:CD, :])

            # u[q, 0:100] = lW @ q ; u[q, 100:200] = rW @ q
            u_ps = psum.tile([P, 2 * CD], F32, name="u_ps", tag="u_ps")
            nc.tensor.matmul(out=u_ps[:, :], lhsT=qt_sb[0:CD, :],
                             rhs=wt_both[0:CD, :], start=True, stop=True)

            # dot products: prod[p,k,c] = ctx[p,k,c] * u_side(k)[p,c]
            prod = work.tile([P, K, CD], F32, name="prod", tag="prod")
            nc.vector.tensor_tensor(
                out=prod[:, 0:lm, :], in0=ceg[:, 1:1 + lm, :],
                in1=u_ps[:, 0:CD].unsqueeze(1).broadcast_to((P, lm, CD)),
                op=Alu.mult)
            nc.vector.tensor_tensor(
                out=prod[:, L:L + rm, :], in0=ceg[:, 1 + L:1 + L + rm, :],
                in1=u_ps[:, CD:2 * CD].unsqueeze(1).broadcast_to((P, rm, CD)),
                op=Alu.mult)
            nc.vector.tensor_reduce(out=dotraw[:, t, 0:lm],
                                    in_=prod[:, 0:lm, :],
                                    axis=Ax.X, op=Alu.add)
            nc.vector.tensor_reduce(out=dotraw[:, t, L:L + rm],
                                    in_=prod[:, L:L + rm, :],
                                    axis=Ax.X, op=Alu.add)

            # row sum-squares over the gathered ranges only: ACT squares,
            # DVE reduces each row
            sq = work.tile([P, KQ, CD], F32, name="sq", tag="sq")
            nc.scalar.activation(out=sq[:, 0:1 + lm, :],
                                 in_=ceg[:, 0:1 + lm, :], func=Act.Square)
            nc.scalar.activation(out=sq[:, 1 + L:1 + L + rm, :],
                                 in_=ceg[:, 1 + L:1 + L + rm, :],
                                 func=Act.Square)
            nc.vector.tensor_reduce(out=qssq[:, t:t + 1], in_=sq[:, 0, :],
                                    axis=Ax.X, op=Alu.add)
            nc.vector.tensor_reduce(out=cssq[:, t, 0:lm],
                                    in_=sq[:, 1:1 + lm, :],
                                    axis=Ax.X, op=Alu.add)
            nc.vector.tensor_reduce(out=cssq[:, t, L:L + rm],
                                    in_=sq[:, 1 + L:1 + L + rm, :],
                                    axis=Ax.X, op=Alu.add)

        def phase2_chunk(t0, t1):
            n = t1 - t0
            ts = slice(t0, t1)
            # 1/max(||row||, 1e-12) for q and ctx rows
            sq_t = stage.tile([P, n], F32, name=f"sqt{t0}")
            nc.scalar.activation(out=sq_t[:, :], in_=qssq[:, ts],
                                 func=Act.Sqrt)
            nc.vector.tensor_scalar(out=sq_t[:, :], in0=sq_t[:, :],
                                    scalar1=1e-12, scalar2=None, op0=Alu.max)
            rq = stage.tile([P, n], F32, name=f"rq{t0}")
            nc.vector.reciprocal(out=rq[:, :], in_=sq_t[:, :])

            csq_t = stage.tile([P, n, K], F32, name=f"csqt{t0}")
            nc.scalar.activation(out=csq_t[:, :, :], in_=cssq[:, ts, :],
                                 func=Act.Sqrt)
            nc.vector.tensor_scalar(out=csq_t[:, :, :], in0=csq_t[:, :, :],
                                    scalar1=1e-12, scalar2=None, op0=Alu.max)
            rctx = stage.tile([P, n, K], F32, name=f"rctx{t0}")
            nc.vector.reciprocal(out=rctx[:, :, :], in_=csq_t[:, :, :])

            scr = stage.tile([P, n, K], F32, name=f"scr{t0}")
            nc.vector.tensor_tensor(out=scr[:, :, :], in0=dotraw[:, ts, :],
                                    in1=rctx[:, :, :], op=Alu.mult)
            nc.vector.tensor_tensor(
                out=scr[:, :, :], in0=scr[:, :, :],
                in1=rq[:, :].unsqueeze(2).broadcast_to((P, n, K)),
                op=Alu.mult)
            nc.vector.tensor_tensor(out=scr[:, :, :], in0=scr[:, :, :],
                                    in1=maskt[:, ts, 1:KQ], op=Alu.add)

            nc.scalar.activation(out=expv[:, ts, :], in_=scr[:, :, :],
                                 func=Act.Exp)

            sum_l = stage.tile([P, n], F32, name=f"suml{t0}")
            nc.vector.tensor_reduce(out=sum_l[:, :], in_=expv[:, ts, 0:L],
                                    axis=Ax.X, op=Alu.add)
            sum_r = stage.tile([P, n], F32, name=f"sumr{t0}")
            nc.vector.tensor_reduce(out=sum_r[:, :], in_=expv[:, ts, L:K],
                                    axis=Ax.X, op=Alu.add)
            rs_l = stage.tile([P, n], F32, name=f"rsl{t0}")
            nc.vector.reciprocal(out=rs_l[:, :], in_=sum_l[:, :])
            rs_r = stage.tile([P, n], F32, name=f"rsr{t0}")
            nc.vector.reciprocal(out=rs_r[:, :], in_=sum_r[:, :])

            # gate logit difference dz = (z1-z0) + (gb1-gb0), where
            # z_j = rs_l * sum_k exp_l[k] gw[j,k] + rs_r * sum_k exp_r[k] gw[j,..]
            d = {}
            gtmp_l = stage.tile([P, n, L], F32, name=f"gtl{t0}")
            gtmp_r = stage.tile([P, n, R], F32, name=f"gtr{t0}")
            for j in (0, 1):
                nc.vector.tensor_tensor(
                    out=gtmp_l[:, :, :], in0=expv[:, ts, 0:L],
                    in1=gwrep[:, j * K:j * K + L].unsqueeze(1)
                        .broadcast_to((P, n, L)),
                    op=Alu.mult)
                d[j, 'l'] = stage.tile([P, n], F32, name=f"d{j}l{t0}")
                nc.vector.tensor_reduce(out=d[j, 'l'][:, :],
                                        in_=gtmp_l[:, :, :],
                                        axis=Ax.X, op=Alu.add)
                nc.vector.tensor_tensor(
                    out=gtmp_r[:, :, :], in0=expv[:, ts, L:K],
                    in1=gwrep[:, j * K + L:(j + 1) * K].unsqueeze(1)
                        .broadcast_to((P, n, R)),
                    op=Alu.mult)
                d[j, 'r'] = stage.tile([P, n], F32, name=f"d{j}r{t0}")
                nc.vector.tensor_reduce(out=d[j, 'r'][:, :],
                                        in_=gtmp_r[:, :, :],
                                        axis=Ax.X, op=Alu.add)

            ddl = stage.tile([P, n], F32, name=f"ddl{t0}")
            nc.vector.tensor_tensor(out=ddl[:, :], in0=d[1, 'l'][:, :],
                                    in1=d[0, 'l'][:, :], op=Alu.subtract)
            ddr = stage.tile([P, n], F32, name=f"ddr{t0}")
            nc.vector.tensor_tensor(out=ddr[:, :], in0=d[1, 'r'][:, :],
                                    in1=d[0, 'r'][:, :], op=Alu.subtract)
            m1 = stage.tile([P, n], F32, name=f"m1{t0}")
            nc.vector.tensor_tensor(out=m1[:, :], in0=ddl[:, :],
                                    in1=rs_l[:, :], op=Alu.mult)
            m2 = stage.tile([P, n], F32, name=f"m2{t0}")
            nc.vector.tensor_tensor(out=m2[:, :], in0=ddr[:, :],
                                    in1=rs_r[:, :], op=Alu.mult)
            dz = stage.tile([P, n], F32, name=f"dz{t0}")
            nc.vector.tensor_tensor(out=dz[:, :], in0=m1[:, :], in1=m2[:, :],
                                    op=Alu.add)
            nc.vector.tensor_scalar(out=dz[:, :], in0=dz[:, :],
                                    scalar1=gbd[:, 0:1], scalar2=None,
                                    op0=Alu.add)

            e1 = stage.tile([P, n], F32, name=f"e1{t0}")
            nc.scalar.activation(out=e1[:, :], in_=dz[:, :], func=Act.Exp)
            den = stage.tile([P, n], F32, name=f"den{t0}")
            nc.vector.tensor_scalar(out=den[:, :], in0=e1[:, :], scalar1=1.0,
                                    scalar2=None, op0=Alu.add)
            rden = stage.tile([P, n], F32, name=f"rden{t0}")
            nc.vector.reciprocal(out=rden[:, :], in_=den[:, :])

            # c_l = g0*rs_l = rs_l/(1+e1); c_r = g1*rs_r = rs_r*e1/(1+e1)
            c_l = stage.tile([P, n], F32, name=f"cl{t0}")
            nc.vector.tensor_tensor(out=c_l[:, :], in0=rs_l[:, :],
                                    in1=rden[:, :], op=Alu.mult)
            c_r = stage.tile([P, n], F32, name=f"cr{t0}")
            nc.vector.tensor_tensor(out=c_r[:, :], in0=rs_r[:, :],
                                    in1=rden[:, :], op=Alu.mult)
            nc.vector.tensor_tensor(out=c_r[:, :], in0=c_r[:, :],
                                    in1=e1[:, :], op=Alu.mult)

            # final per-slot weights
            nc.vector.tensor_tensor(
                out=wall[:, ts, 0:L], in0=expv[:, ts, 0:L],
                in1=c_l[:, :].unsqueeze(2).broadcast_to((P, n, L)),
                op=Alu.mult)
            nc.vector.tensor_tensor(
                out=wall[:, ts, L:K], in0=expv[:, ts, L:K],
                in1=c_r[:, :].unsqueeze(2).broadcast_to((P, n, R)),
                op=Alu.mult)

        def phase3_tile(t):
            lm, rm = lmax[t], rmax[t]
            slots = list(range(0, lm)) + list(range(L, L + rm))
            wvg = wvp.tile([P, K, WD], F32, name="wvg", tag="wvg")
            for s in slots:
                nc.gpsimd.indirect_dma_start(
                    out=wvg[:, s, :], out_offset=None,
                    in_=wv[:, :],
                    in_offset=bass.IndirectOffsetOnAxis(
                        ap=wcl[:, t, s:s + 1], axis=0),
                )
            acc_a = work.tile([P, WD], F32, name="acc_a", tag="acc_a")
            acc_b = work.tile([P, WD], F32, name="acc_b", tag="acc_b")
            nc.vector.tensor_scalar(out=acc_a[:, :], in0=wvg[:, slots[0], :],
                                    scalar1=wall[:, t, slots[0]:slots[0] + 1],
                                    scalar2=None, op0=Alu.mult)
            src, dst = acc_a, acc_b
            for s in slots[1:]:
                nc.vector.scalar_tensor_tensor(
                    out=dst[:, :], in0=wvg[:, s, :],
                    scalar=wall[:, t, s:s + 1], in1=src[:, :],
                    op0=Alu.mult, op1=Alu.add)
                src, dst = dst, src
            nc.sync.dma_start(out=out[t * P:(t + 1) * P, :], in_=src[:, :])

        for c0 in range(0, NT, CHUNK):
            for t in range(c0, c0 + CHUNK):
                phase1_tile(t)
            phase2_chunk(c0, c0 + CHUNK)
            for t in range(c0, c0 + CHUNK):
                phase3_tile(t)

    nc.compile()
    return nc


_NC_CACHE = {}


def _get_nc(lmax, rmax):
    key = (tuple(lmax), tuple(rmax))
    if key not in _NC_CACHE:
        _NC_CACHE[key] = _build_nc(list(lmax), list(rmax))
    return _NC_CACHE[key]


def _plan(llen, rlen, search_seconds=40.0, max_evals=250000):
    """Pick the query order (host-side index marshalling) minimizing the
    gather-instruction count, and derive per-tile slot counts.

    Queries are sorted globally and dealt round-robin to cores, so core c's
    tile t holds global sorted positions [1024t, 1024(t+1)) with stride 8 --
    every core shares the same per-tile length profile. Queries are grouped
    into (llen, rlen) cells; a bounded hill-climb over the cell order packs
    cells so per-tile (max llen + max rlen) is small.
    """
    import time as _time

    GT = B // NT  # global queries per tile row
    cells = {}
    for i, (a, b) in enumerate(zip(llen.tolist(), rlen.tolist())):
        cells.setdefault((a, b), []).append(i)
    cells = {k: np.asarray(v) for k, v in cells.items()}
    keys = list(cells)
    sizes = {k: len(v) for k, v in cells.items()}

    def cost_fast(co):
        # walk cells in order, cutting tiles every GT queries; a tile costs
        # 1 (qidx gather) + 2*(max llen + max rlen)  [ce + wv instructions]
        rem, cl, cr, tot = GT, 0, 0, 0
        for ab in co:
            n = sizes[ab]
            while n > 0:
                take = min(n, rem)
                if ab[0] > cl:
                    cl = ab[0]
                if ab[1] > cr:
                    cr = ab[1]
                n -= take
                rem -= take
                if rem == 0:
                    tot += 1 + 2 * (cl + cr)
                    rem, cl, cr = GT, 0, 0
        if rem < GT:
            tot += 1 + 2 * (cl + cr)
        return tot

    best_co = sorted(keys, key=lambda ab: -max(ab[0] * 10, ab[1] * 7))
    best = cost_fast(best_co)
    t0 = _time.time()
    if len(keys) >= 2:
        for seed in range(4):
            rng = np.random.default_rng(seed)
            co, cur = list(best_co) if seed == 0 else sorted(
                keys, key=lambda ab: -(ab[0] * 10 + ab[1] * 7)), None
            cur = cost_fast(co)
            for _ in range(max_evals):
                if _time.time() - t0 > search_seconds:
                    break
                i, j = rng.integers(0, len(keys), 2)
                if i == j:
                    continue
                co2 = list(co)
                co2[i], co2[j] = co2[j], co2[i]
                c = cost_fast(co2)
                if c <= cur:
                    co, cur = co2, c
                    if c < best:
                        best, best_co = c, list(co)

    order = np.concatenate([cells[ab] for ab in best_co])
    ls, rs = llen[order], rlen[order]
    lm = [int(ls[t * GT:(t + 1) * GT].max()) for t in range(NT)]
    rm = [int(rs[t * GT:(t + 1) * GT].max()) for t in range(NT)]
    return order, lm, rm


def kernel(**inputs):
    inp = {k: np.asarray(v) for k, v in inputs.items()}
    llen = (inp["lixs_c"] != -1).sum(axis=1).astype(np.int64)
    rlen = (inp["rixs_c"] != -1).sum(axis=1).astype(np.int64)
    order, lmax, rmax = _plan(llen, rlen)

    nc = _get_nc(lmax, rmax)
    shared = {k: inp[k] for k in ("ce_raw", "wvec", "lW", "rW", "gL_w", "gL_b")}
    in_maps = []
    for c in range(N_CORES):
        # core c takes global sorted positions c, c+8, c+16, ...
        rows = order[c::N_CORES]
        m = dict(shared)
        for name in ("qidx", "lixs_c", "rixs_c", "lixs_w", "rixs_w"):
            m[name] = np.ascontiguousarray(inp[name][rows])
        in_maps.append(m)
    res = run_bass_kernel_spmd(nc, in_maps, list(range(N_CORES)))
    out_sorted = np.empty((B, WD), np.float32)
    for c in range(N_CORES):
        out_sorted[c::N_CORES] = res.results[c]["out"]
    out = np.empty((B, WD), np.float32)
    out[order] = out_sorted
    return out
